# revision 1
# baseline (speedup 1.0000x reference)
"""SS2D CrossBlock kernel for 8 NeuronCores (Trainium2).

Sharding: core c handles (b = c//2, d-half = c%2). Each core computes the
full pre-scan pipeline for its batch b (in_proj, depthwise conv, x_dbl
projections shared across the pair), then scans all 4 directions for its
96-channel half, combines directions locally, and finishes LN + gate +
out_proj with a tiny pair AllReduce for the LN statistics. Host sums the
two partial out_proj results per batch.
"""
import numpy as np
import ml_dtypes
from contextlib import ExitStack
BF_NP = np.float16

import concourse.bass as bass
import concourse.bacc as bacc_mod
import concourse.tile as tile
from concourse import mybir
from concourse.bass_utils import run_bass_kernel_spmd

F32 = mybir.dt.float32
BF = mybir.dt.float16
AF = mybir.ActivationFunctionType
OP = mybir.AluOpType

B, HH, WW, DM = 4, 32, 32, 96
DI, NS, RD, K, L = 192, 16, 6, 4, 1024
DH = 96            # channels per core (d-half)
NT = DH // 8       # 12 scan tiles per direction (8 d x 16 n = 128 rows)
EPS = 1e-5

_NC = None


def nat3(ap):
    return ap.rearrange("p (a b) -> p a b", a=32, b=32)


def tview(ap):
    # tview(X)[p, w, h] = X[p, h*32 + w]
    return ap.rearrange("p (h w) -> p w h", h=32, w=32)


def build():
    nc = bacc_mod.Bacc(trn_type="TRN2", target_bir_lowering=False,
                       debug=False, num_devices=8)

    def din(name, shape):
        return nc.dram_tensor(name, shape, F32, kind="ExternalInput")

    def dbf(name, shape):
        return nc.dram_tensor(name, shape, BF, kind="ExternalInput")

    xT = dbf("xT", [DM, L])                  # x[b] transposed (dm, l)
    w_xi = dbf("w_xi", [DM, DI])             # in_proj lhsT for xi (2x96 blocks)
    w_z = dbf("w_z", [DM, DH])               # in_proj lhsT for this core's z
    convw = din("convw", [DH, 2 * 9])        # per-channel taps, 2 halves
    convb = din("convb", [DH, 2])
    xpw = dbf("xpw", [DH, K * 2 * 64])       # x_dbl lhsT packed (rows 0:6 dts, 32:64 B,C)
    dtw = dbf("dtw", [RD, K * DH])           # dt lhsT per k: [6, 96]
    dtb = din("dtb", [DH, K])                # dt bias per k (col k)
    app = din("app", [128, K * NT])          # exp scale A rows per (k,t)
    bcm = dbf("bcm", [DH, NT * 128])         # broadcast 0/1 lhsT per t
    red = dbf("red", [128, NT * DH])         # hC reduce lhsT per t
    dsum = din("dsum", [DH, 1])              # sum_k Ds
    gam = din("gam", [DH, 1])
    bet = din("bet", [DH, 1])
    wout = dbf("wout", [DH, DM])             # out_proj lhsT slice
    ones96 = dbf("ones96", [DH, 2])          # col0: ones (y), col1: ones (y2)
    sel2 = din("sel2", [2, 2 * DH])          # mu/inv row-select lhsT

    out_part = nc.dram_tensor("out_part", [DM, L], F32, kind="ExternalOutput")

    stats_in = nc.dram_tensor("stats_in", [2, L], F32)
    stats_out = nc.dram_tensor("stats_out", [2, L], F32)
    minv_dram = nc.dram_tensor("minv_dram", [2, L], F32)
    groups = [[0, 1], [2, 3], [4, 5], [6, 7]]

    with tile.TileContext(nc) as tc, ExitStack() as ctx:
        wpool = ctx.enter_context(tc.tile_pool(name="w", bufs=1))
        spool = ctx.enter_context(tc.tile_pool(name="s", bufs=1))
        kpool = ctx.enter_context(tc.tile_pool(name="kk", bufs=2))
        k1pool = ctx.enter_context(tc.tile_pool(name="k1", bufs=1))
        tpool = ctx.enter_context(tc.tile_pool(name="t", bufs=2))
        ppool = ctx.enter_context(tc.tile_pool(name="pp", bufs=1, space="PSUM"))
        bpool = ctx.enter_context(tc.tile_pool(name="bb", bufs=1, space="PSUM"))
        ypool = ctx.enter_context(tc.tile_pool(name="yy", bufs=1, space="PSUM"))

        def load(shape, src, name, dt=F32):
            t = wpool.tile(shape, dt, tag=name, name=name + "_sb")
            nc.sync.dma_start(t[:], src[:])
            return t

        # ---- weight loads ----
        w_xi_sb = load([DM, DI], w_xi, "w_xi", BF)
        w_z_sb = load([DM, DH], w_z, "w_z", BF)
        convw_sb = load([DH, 2 * 9], convw, "convw")
        convb_sb = load([DH, 2], convb, "convb")
        xpw_sb = load([DH, K * 2 * 64], xpw, "xpw", BF)
        dtw_sb = load([RD, K * DH], dtw, "dtw", BF)
        dtb_sb = load([DH, K], dtb, "dtb")
        app_sb = load([128, K * NT], app, "app")
        bcm_sb = load([DH, NT * 128], bcm, "bcm", BF)
        red_sb = load([128, NT * DH], red, "red", BF)
        dsum_sb = load([DH, 1], dsum, "dsum")
        gam_sb = load([DH, 1], gam, "gam")
        bet_sb = load([DH, 1], bet, "bet")
        wout_sb = load([DH, DM], wout, "wout", BF)
        ones_sb = load([DH, 2], ones96, "ones96", BF)
        sel2_sb = load([2, 2 * DH], sel2, "sel2")
        xT_sb = load([DM, L], xT, "xTs", BF)

        # ---- phase 1: in_proj ----
        # xi (2 x 96-row tiles) and z for this half; contraction over DM=96
        PADL = 34 * 34 + 4   # +4 so the (2,2) tap 32x34 window slice stays in-bounds
        sg = spool.tile([DH, L], BF)
        xpad = [spool.tile([DH, PADL], BF, name=f"xpad{i}") for i in range(2)]
        for cblk in range(2):
            nc.vector.memset(xpad[cblk][:], 0.0)
        pp = 0
        for cblk in range(2):
            for h in range(2):
                ps = ppool.tile([DH, 512], F32, tag=f"ping{pp % 2}",
                                name=f"xi{cblk}{h}")
                pp += 1
                nc.tensor.matmul(ps[:],
                                 w_xi_sb[:, cblk * DH:(cblk + 1) * DH],
                                 xT_sb[:, h * 512:(h + 1) * 512],
                                 start=True, stop=True)
                dst = xpad[cblk][:, 35:35 + 32 * 34]
                dstv = dst.rearrange("p (r c) -> p r c", r=32, c=34)[:, :, 0:32]
                half = dstv[:, h * 16:(h + 1) * 16, :]
                src = ps[:].rearrange("p (r c) -> p r c", r=16, c=32)
                nc.scalar.activation(half, src, AF.Copy)
        zt = spool.tile([DH, L], BF)
        for h in range(2):
            ps = ppool.tile([DH, 512], F32, tag=f"ping{pp % 2}", name=f"z{h}")
            pp += 1
            nc.tensor.matmul(ps[:], w_z_sb[:],
                             xT_sb[:, h * 512:(h + 1) * 512],
                             start=True, stop=True)
            nc.scalar.activation(zt[:, h * 512:(h + 1) * 512], ps[:], AF.Copy)
            nc.scalar.activation(sg[:, h * 512:(h + 1) * 512], ps[:], AF.Sigmoid)
        nc.vector.tensor_tensor(sg[:], sg[:], zt[:], OP.mult)

        # ---- phase 2: depthwise conv + silu -> xc ----
        xc = [spool.tile([DH, L], BF, name=f"xc{i}") for i in range(2)]
        for cblk in range(2):
            acc = kpool.tile([DH, L], BF, tag="cacc")
            for tap in range(9):
                dy, dx = tap // 3, tap % 3
                view = xpad[cblk][:, dy * 34 + dx:dy * 34 + dx + 32 * 34]
                view = view.rearrange("p (r c) -> p r c", r=32, c=34)[:, :, 0:32]
                wcol = convw_sb[:, cblk * 9 + tap:cblk * 9 + tap + 1]
                if tap == 0:
                    bcol = convb_sb[:, cblk:cblk + 1]
                    nc.vector.tensor_scalar(nat3(acc[:]), view, wcol, bcol,
                                            OP.mult, OP.add)
                else:
                    acc2 = kpool.tile([DH, L], BF, tag="cacc")
                    nc.vector.scalar_tensor_tensor(
                        nat3(acc2[:]), view, wcol, nat3(acc[:]), OP.mult, OP.add)
                    acc = acc2
            nc.scalar.activation(xc[cblk][:], acc[:], AF.Sigmoid)
            nc.vector.tensor_tensor(xc[cblk][:], xc[cblk][:], acc[:], OP.mult)

        # ---- phase 3: x_dbl, dt, delta, u  (row-major for all k) ----
        du = []     # [DH, 2048] per k: cols 0:1024 delta, 1024:2048 u
        bc_sb = []  # [2*NS, L] per k: B rows then C rows
        for k in range(K):
            zk = ppool.tile([64, 512], F32, tag="ping0", name="zk")
            zk2 = ppool.tile([64, 512], F32, tag="ping1", name="zk2")
            for h, zz in enumerate((zk, zk2)):
                for cblk in range(2):
                    w0 = (k * 2 + cblk) * 64
                    nc.tensor.matmul(
                        zz[:],
                        xpw_sb[:, w0:w0 + 64],
                        xc[cblk][:, h * 512:(h + 1) * 512],
                        start=(cblk == 0), stop=(cblk == 1))
            dts = kpool.tile([RD, L], BF, tag="dts")
            bck = k1pool.tile([2 * NS, L], BF, tag=f"bck{k}")
            for h, zz in enumerate((zk, zk2)):
                nc.scalar.activation(dts[:, h * 512:(h + 1) * 512],
                                     zz[0:RD, :], AF.Copy)
                nc.vector.tensor_copy(bck[:, h * 512:(h + 1) * 512],
                                      zz[32:64, :])
            bc_sb.append(bck)

            dtd = ppool.tile([DH, 512], F32, tag="ping0", name="dtd")
            dtd2 = ppool.tile([DH, 512], F32, tag="ping1", name="dtd2")
            for h, dd in enumerate((dtd, dtd2)):
                nc.tensor.matmul(dd[:], dtw_sb[:, k * DH:(k + 1) * DH],
                                 dts[:, h * 512:(h + 1) * 512],
                                 start=True, stop=True)
            duk = k1pool.tile([DH, 2 * L], BF, tag=f"du{k}")
            esp = kpool.tile([DH, L], F32, tag="esp")
            for h, dd in enumerate((dtd, dtd2)):
                nc.scalar.activation(esp[:, h * 512:(h + 1) * 512], dd[:],
                                     AF.Exp, bias=dtb_sb[:, k:k + 1], scale=1.0)
            # delta = ln(1 + e^(dt+bias)) ; store row-major
            nc.scalar.activation(duk[:, 0:L], esp[:], AF.Ln, bias=1.0, scale=1.0)
            # u = delta * xs_k (xs_k = xc permuted; delta is row-major here,
            # so u is row-major too: u_rm[l] = delta_rm[l] * xc[l])
            du.append(duk)

        # u needs xc rows for THIS core's half: xc half spans one tile
        # (dh=0 -> xc[0], dh=1 -> xc[1]) selected on host via weight layout?
        # No: xc tiles are global channels; this core's d-half is xc[dh].
        # dh is baked per-core via the dtw/xpw slices? xc itself is global.
        # We pass dh through a dedicated input row select: simplest is that
        # u uses xc[dh] — dh is a host-side constant per core, but the BIR
        # is shared across cores (SPMD). So both halves' xc are present;
        # we need xc_half = xc[dh]. Use partition_id-free approach: the
        # dt/delta path above already used per-core weights (dtw slice),
        # and xc half selection is done via a per-core input matrix:
        # halfsel [DI, DH] 0/1 -> PE copy. Cheap: 2 matmuls per k? Instead
        # we select on host by reordering w_xi so that xc[0] is ALWAYS this
        # core's half and xc[1] the other half. x_dbl contraction covers
        # both halves regardless (xpw rows reordered to match).
        for k in range(K):
            nc.vector.tensor_tensor(du[k][:, L:2 * L], du[k][:, 0:L],
                                    xc[0][:], OP.mult)

        # ---- phase 4: per-direction scan ----
        y_ps = ypool.tile([DH, L], F32)
        for k in range(K):
            trans = k in (1, 3)
            flip = k >= 2
            bb = kpool.tile([128, L], BF, tag="Bb")
            cb = kpool.tile([128, L], BF, tag="Cb")
            for r in range(8):
                nc.sync.dma_start(bb[16 * r:16 * (r + 1), :],
                                  bc_sb[k][0:NS, :])
                nc.sync.dma_start(cb[16 * r:16 * (r + 1), :],
                                  bc_sb[k][NS:2 * NS, :])
            for t in range(NT):
                bcp = bpool.tile([128, 2 * L], F32, tag="bcp")
                for q in range(4):
                    nc.tensor.matmul(bcp[:, q * 512:(q + 1) * 512],
                                     bcm_sb[:, t * 128:(t + 1) * 128],
                                     du[k][:, q * 512:(q + 1) * 512],
                                     start=True, stop=True)
                a_t = tpool.tile([128, L], BF, tag="a")
                b_t = tpool.tile([128, L], BF, tag="b")
                scl = app_sb[:, k * NT + t:k * NT + t + 1]
                if trans:
                    nc.scalar.activation(tview(a_t[:]), nat3(bcp[:, 0:L]),
                                         AF.Exp, bias=0.0, scale=scl)
                    nc.vector.scalar_tensor_tensor(
                        tview(b_t[:]), nat3(bcp[:, L:2 * L]), 1.0,
                        nat3(bb[:]), OP.mult, OP.mult)
                else:
                    nc.scalar.activation(a_t[:], bcp[:, 0:L],
                                         AF.Exp, bias=0.0, scale=scl)
                    nc.vector.scalar_tensor_tensor(
                        b_t[:], bcp[:, L:2 * L], 1.0, bb[:], OP.mult, OP.mult)
                h_t = tpool.tile([128, L], BF, tag="h")
                if flip:
                    nc.vector.tensor_tensor_scan(
                        h_t[:, ::-1], a_t[:, ::-1], b_t[:, ::-1], 0.0,
                        OP.mult, OP.add)
                else:
                    nc.vector.tensor_tensor_scan(
                        h_t[:], a_t[:], b_t[:], 0.0, OP.mult, OP.add)
                hc_t = tpool.tile([128, L], BF, tag="hc")
                if trans:
                    nc.vector.tensor_tensor(nat3(hc_t[:]), tview(h_t[:]),
                                            nat3(cb[:]), OP.mult)
                else:
                    nc.vector.tensor_tensor(hc_t[:], h_t[:], cb[:], OP.mult)
                for q in range(2):
                    nc.tensor.matmul(y_ps[:, q * 512:(q + 1) * 512],
                                     red_sb[:, t * DH:(t + 1) * DH],
                                     hc_t[:, q * 512:(q + 1) * 512],
                                     start=(k == 0 and t == 0),
                                     stop=(k == 3 and t == NT - 1))

        # ---- phase 5: D-term, LN stats, AllReduce ----
        y_full = spool.tile([DH, L], BF)
        nc.vector.scalar_tensor_tensor(y_full[:], xc[0][:], dsum_sb[:],
                                       y_ps[:], OP.mult, OP.add)
        y2 = spool.tile([DH, L], BF)
        nc.scalar.activation(y2[:], y_full[:], AF.Square)
        st_y = spool.tile([1, L], F32)
        st_y2 = spool.tile([1, L], F32)
        for h in range(2):
            for row, (src_t, dst_t) in enumerate(((y_full, st_y), (y2, st_y2))):
                ssp = ppool.tile([1, 512], F32, tag=f"ping{(2 * h + row) % 2}",
                                 name=f"st{h}{row}")
                nc.tensor.matmul(ssp[:], ones_sb[:, row:row + 1],
                                 src_t[:, h * 512:(h + 1) * 512],
                                 start=True, stop=True)
                nc.scalar.activation(dst_t[:, h * 512:(h + 1) * 512],
                                     ssp[:], AF.Copy)
        nc.sync.dma_start(stats_in[0:1, :], st_y[:])
        nc.sync.dma_start(stats_in[1:2, :], st_y2[:])
        nc.gpsimd.collective_compute(
            "AllReduce", OP.add, replica_groups=groups,
            ins=[stats_in[:]], outs=[stats_out[:]])
        # reshape [2, 1024] -> [128, 16] straight from DRAM
        st128 = spool.tile([128, 16], F32)
        nc.sync.dma_start(
            st128[:].rearrange("p (s f) -> p s f", s=2, f=8),
            stats_out[:].rearrange("s (p f) -> p s f", p=128, f=8))
        mu8 = spool.tile([128, 8], F32)
        nc.scalar.activation(mu8[:], st128[:, 0:8], AF.Copy, scale=1.0 / DI)
        msq = spool.tile([128, 8], F32)
        nc.scalar.activation(msq[:], st128[:, 0:8], AF.Square, scale=1.0 / DI)
        var8 = spool.tile([128, 8], F32)
        nc.vector.scalar_tensor_tensor(var8[:], st128[:, 8:16], 1.0 / DI,
                                       msq[:], OP.mult, OP.subtract)
        eps_sb = spool.tile([128, 1], F32)
        nc.vector.memset(eps_sb[:], EPS)
        sd8 = spool.tile([128, 8], F32)
        nc.scalar.activation(sd8[:], var8[:], AF.Sqrt, bias=eps_sb[:], scale=1.0)
        inv8 = spool.tile([128, 8], F32)
        nc.vector.reciprocal(inv8[:], sd8[:])
        nc.sync.dma_start(
            minv_dram[0:1, :].rearrange("s (p f) -> p s f", p=128, f=8),
            mu8[:].unsqueeze(1))
        nc.sync.dma_start(
            minv_dram[1:2, :].rearrange("s (p f) -> p s f", p=128, f=8),
            inv8[:].unsqueeze(1))
        minv = spool.tile([2, L], F32)
        nc.sync.dma_start(minv[:], minv_dram[:])

        # broadcast mu/inv across partitions via PE (contraction dim 2)
        t1 = spool.tile([DH, L], BF)
        for h in range(2):
            mm = ppool.tile([DH, 512], F32, tag=f"ping{h % 2}", name=f"mu{h}")
            nc.tensor.matmul(mm[:], sel2_sb[:, 0:DH],
                             minv[:, h * 512:(h + 1) * 512],
                             start=True, stop=True)
            nc.vector.tensor_tensor(t1[:, h * 512:(h + 1) * 512],
                                    y_full[:, h * 512:(h + 1) * 512],
                                    mm[:], OP.subtract)
        t2 = spool.tile([DH, L], BF)
        for h in range(2):
            ii = ppool.tile([DH, 512], F32, tag=f"ping{h % 2}", name=f"iv{h}")
            nc.tensor.matmul(ii[:], sel2_sb[:, DH:2 * DH],
                             minv[:, h * 512:(h + 1) * 512],
                             start=True, stop=True)
            nc.vector.scalar_tensor_tensor(t2[:, h * 512:(h + 1) * 512],
                                           t1[:, h * 512:(h + 1) * 512],
                                           gam_sb[:], ii[:], OP.mult, OP.mult)
        t3 = spool.tile([DH, L], BF)
        nc.vector.scalar_tensor_tensor(t3[:], t2[:], bet_sb[:], sg[:],
                                       OP.add, OP.mult)

        o_sb = spool.tile([DM, L], F32)
        for h in range(2):
            oo = ppool.tile([DM, 512], F32, tag=f"ping{h % 2}", name=f"o{h}")
            nc.tensor.matmul(oo[:], wout_sb[:],
                             t3[:, h * 512:(h + 1) * 512],
                             start=True, stop=True)
            nc.scalar.activation(o_sb[:, h * 512:(h + 1) * 512], oo[:], AF.Copy)
        nc.sync.dma_start(out_part[:], o_sb[:])

    nc.finalize()
    return nc


def _prep_inputs(inputs):
    """Build the 8 per-core input maps. Core c: b = c//2, dh = c%2."""
    x = np.asarray(inputs["x"], np.float32)
    in_proj_w = np.asarray(inputs["in_proj_w"], np.float32)
    conv_w = np.asarray(inputs["conv_w"], np.float32)
    conv_b = np.asarray(inputs["conv_b"], np.float32)
    xpw = np.asarray(inputs["x_proj_weight"], np.float32)
    dtw = np.asarray(inputs["dt_projs_weight"], np.float32)
    dtb = np.asarray(inputs["dt_projs_bias"], np.float32)
    A_logs = np.asarray(inputs["A_logs"], np.float32)
    Ds = np.asarray(inputs["Ds"], np.float32)
    gam = np.asarray(inputs["ln_gamma"], np.float32)
    bet = np.asarray(inputs["ln_beta"], np.float32)
    wout = np.asarray(inputs["out_proj_w"], np.float32)

    xTf = x.reshape(B, L, DM).transpose(0, 2, 1).copy()      # (B, 96, 1024)
    w_in_T = in_proj_w.T.copy()                               # (96, 384)
    convw9 = conv_w.reshape(DI, 9)                            # (192, 9)
    A = -np.exp(A_logs).reshape(K, DI, NS)                    # (K, 192, 16)
    Dsum_full = Ds.reshape(K, DI).sum(0)                      # (192,)

    bcm = np.zeros((DH, NT * 128), np.float32)
    for t in range(NT):
        for j in range(128):
            bcm[8 * t + j // 16, t * 128 + j] = 1.0
    red = np.zeros((128, NT * DH), np.float32)
    for t in range(NT):
        for j in range(128):
            red[j, t * DH + 8 * t + j // 16] = 1.0
    ones96 = np.ones((DH, 2), np.float32)
    sel2 = np.zeros((2, 2 * DH), np.float32)
    sel2[0, 0:DH] = 1.0
    sel2[1, DH:2 * DH] = 1.0

    in_maps = []
    for c in range(8):
        b, dh = c // 2, c % 2
        ds = slice(dh * DH, (dh + 1) * DH)
        other = slice((1 - dh) * DH, (2 - dh) * DH)
        # xc tile 0 must hold THIS core's half: reorder in_proj rows and
        # x_dbl contraction rows to match (half-first ordering).
        w_xi = np.concatenate([w_in_T[:, ds], w_in_T[:, other]], axis=1)
        convw_r = np.concatenate([convw9[ds], convw9[other]], axis=1)
        convb_r = np.stack([conv_b[ds], conv_b[other]], axis=1)
        xpw_r = np.zeros((DH, K * 2 * 64), np.float32)
        for k in range(K):
            wk = xpw[k].T  # (192, 38)
            for cblk, sl in enumerate((ds, other)):
                w0 = (k * 2 + cblk) * 64
                xpw_r[:, w0:w0 + RD] = wk[sl][:, 0:RD]
                xpw_r[:, w0 + 32:w0 + 64] = wk[sl][:, RD:RD + 2 * NS]
        dtw_r = np.zeros((RD, K * DH), np.float32)
        for k in range(K):
            dtw_r[:, k * DH:(k + 1) * DH] = dtw[k, ds, :].T
        dtb_r = dtb.reshape(K, DI)[:, ds].T.copy()            # (96, K)
        app = np.zeros((128, K * NT), np.float32)
        for k in range(K):
            for t in range(NT):
                for j in range(128):
                    app[j, k * NT + t] = A[k, dh * DH + 8 * t + j // 16, j % 16]
        in_maps.append({
            "xT": xTf[b].astype(BF_NP),
            "w_xi": w_xi.astype(BF_NP),
            "w_z": w_in_T[:, DI + dh * DH: DI + (dh + 1) * DH].astype(BF_NP),
            "convw": convw_r,
            "convb": convb_r,
            "xpw": xpw_r.astype(BF_NP),
            "dtw": dtw_r.astype(BF_NP),
            "dtb": dtb_r,
            "app": app,
            "bcm": bcm.astype(BF_NP),
            "red": red.astype(BF_NP),
            "dsum": Dsum_full[ds][:, None],
            "gam": gam[ds][:, None],
            "bet": bet[ds][:, None],
            "wout": wout[:, ds].T.astype(BF_NP),
            "ones96": ones96.astype(BF_NP),
            "sel2": sel2,
        })
    return in_maps


def kernel(**inputs):
    global _NC
    if _NC is None:
        _NC = build()
    in_maps = _prep_inputs(inputs)
    res = run_bass_kernel_spmd(_NC, in_maps, list(range(8)))
    out = np.zeros((B, L, DM), np.float32)
    for b in range(B):
        part = res.results[2 * b]["out_part"] + res.results[2 * b + 1]["out_part"]
        out[b] = part.T
    return out.reshape(B, HH, WW, DM)



# revision 4
# speedup vs baseline: 1.4613x; 1.4613x over previous
"""SS2D CrossBlock kernel for 8 NeuronCores (Trainium2).

Sharding: core c handles (b = c//2, d-half = c%2). Each core computes the
full pre-scan pipeline for its batch b (in_proj, depthwise conv, x_dbl
projections shared across the pair), then scans all 4 directions for its
96-channel half, combines directions locally, and finishes LN + gate +
out_proj with a tiny pair AllReduce for the LN statistics. Host sums the
two partial out_proj results per batch.
"""
import numpy as np
import ml_dtypes
from contextlib import ExitStack
BF_NP = np.float16

import concourse.bass as bass
import concourse.bacc as bacc_mod
import concourse.tile as tile
from concourse import mybir
from concourse.bass_utils import run_bass_kernel_spmd

F32 = mybir.dt.float32
BF = mybir.dt.float16
AF = mybir.ActivationFunctionType
OP = mybir.AluOpType

B, HH, WW, DM = 4, 32, 32, 96
DI, NS, RD, K, L = 192, 16, 6, 4, 1024
DH = 96            # channels per core (d-half)
NT = DH // 8       # 12 scan tiles per direction (8 d x 16 n = 128 rows)
EPS = 1e-5

_NC = None


def nat3(ap):
    return ap.rearrange("p (a b) -> p a b", a=32, b=32)


def tview(ap):
    # tview(X)[p, w, h] = X[p, h*32 + w]
    return ap.rearrange("p (h w) -> p w h", h=32, w=32)


def build():
    nc = bacc_mod.Bacc(trn_type="TRN2", target_bir_lowering=False,
                       debug=False, num_devices=8)

    def din(name, shape):
        return nc.dram_tensor(name, shape, F32, kind="ExternalInput")

    def dbf(name, shape):
        return nc.dram_tensor(name, shape, BF, kind="ExternalInput")

    xT = dbf("xT", [DM, L])                  # x[b] transposed (dm, l)
    w_xi = dbf("w_xi", [DM, DI])             # in_proj lhsT for xi (2x96 blocks)
    w_z = dbf("w_z", [DM, DH])               # in_proj lhsT for this core's z
    convw = din("convw", [DH, 2 * 9])        # per-channel taps, 2 halves
    convb = din("convb", [DH, 2])
    xpw = dbf("xpw", [DH, K * 2 * 64])       # x_dbl lhsT packed (rows 0:6 dts, 32:64 B,C)
    dtw = dbf("dtw", [RD, K * DH])           # dt lhsT per k: [6, 96]
    dtb = din("dtb", [DH, K])                # dt bias per k (col k)
    app = din("app", [128, K * NT])          # exp scale A rows per (k,t)
    bcm = dbf("bcm", [DH, NT * 128])         # broadcast 0/1 lhsT per t
    red = dbf("red", [128, NT * DH])         # hC reduce lhsT per t
    dsum = din("dsum", [DH, 1])              # sum_k Ds
    gam = din("gam", [DH, 1])
    bet = din("bet", [DH, 1])
    wout = dbf("wout", [DH, DM])             # out_proj lhsT slice
    ones96 = dbf("ones96", [DH, 2])          # col0: ones (y), col1: ones (y2)
    sel2 = din("sel2", [2, 2 * DH])          # mu/inv row-select lhsT

    out_part = nc.dram_tensor("out_part", [DM, L], F32, kind="ExternalOutput")

    stats_in = nc.dram_tensor("stats_in", [2, L], F32)
    stats_out = nc.dram_tensor("stats_out", [2, L], F32)
    minv_dram = nc.dram_tensor("minv_dram", [2, L], F32)
    groups = [[0, 1], [2, 3], [4, 5], [6, 7]]

    with tile.TileContext(nc) as tc, ExitStack() as ctx:
        wpool = ctx.enter_context(tc.tile_pool(name="w", bufs=1))
        spool = ctx.enter_context(tc.tile_pool(name="s", bufs=1))
        kpool = ctx.enter_context(tc.tile_pool(name="kk", bufs=2))
        k1pool = ctx.enter_context(tc.tile_pool(name="k1", bufs=1))
        tpool = ctx.enter_context(tc.tile_pool(name="t", bufs=2))
        ppool = ctx.enter_context(tc.tile_pool(name="pp", bufs=1, space="PSUM"))
        bpool = ctx.enter_context(tc.tile_pool(name="bb", bufs=1, space="PSUM"))
        ypool = ctx.enter_context(tc.tile_pool(name="yy", bufs=1, space="PSUM"))

        def load(shape, src, name, dt=F32):
            t = wpool.tile(shape, dt, tag=name, name=name + "_sb")
            nc.sync.dma_start(t[:], src[:])
            return t

        # ---- weight loads ----
        w_xi_sb = load([DM, DI], w_xi, "w_xi", BF)
        w_z_sb = load([DM, DH], w_z, "w_z", BF)
        convw_sb = load([DH, 2 * 9], convw, "convw")
        convb_sb = load([DH, 2], convb, "convb")
        xpw_sb = load([DH, K * 2 * 64], xpw, "xpw", BF)
        dtw_sb = load([RD, K * DH], dtw, "dtw", BF)
        dtb_sb = load([DH, K], dtb, "dtb")
        app_sb = load([128, K * NT], app, "app")
        bcm_sb = load([DH, NT * 128], bcm, "bcm", BF)
        red_sb = load([128, NT * DH], red, "red", BF)
        dsum_sb = load([DH, 1], dsum, "dsum")
        gam_sb = load([DH, 1], gam, "gam")
        bet_sb = load([DH, 1], bet, "bet")
        wout_sb = load([DH, DM], wout, "wout", BF)
        ones_sb = load([DH, 2], ones96, "ones96", BF)
        sel2_sb = load([2, 2 * DH], sel2, "sel2")
        xT_sb = load([DM, L], xT, "xTs", BF)

        # ---- phase 1: in_proj ----
        # xi (2 x 96-row tiles) and z for this half; contraction over DM=96
        PADL = 34 * 34 + 4   # +4 so the (2,2) tap 32x34 window slice stays in-bounds
        sg = spool.tile([DH, L], BF)
        xpad = [spool.tile([DH, PADL], BF, name=f"xpad{i}") for i in range(2)]
        for cblk in range(2):
            nc.vector.memset(xpad[cblk][:], 0.0)
        pp = 0
        for cblk in range(2):
            for h in range(2):
                ps = ppool.tile([DH, 512], F32, tag=f"ping{pp % 2}",
                                name=f"xi{cblk}{h}")
                pp += 1
                nc.tensor.matmul(ps[:],
                                 w_xi_sb[:, cblk * DH:(cblk + 1) * DH],
                                 xT_sb[:, h * 512:(h + 1) * 512],
                                 start=True, stop=True)
                dst = xpad[cblk][:, 35:35 + 32 * 34]
                dstv = dst.rearrange("p (r c) -> p r c", r=32, c=34)[:, :, 0:32]
                half = dstv[:, h * 16:(h + 1) * 16, :]
                src = ps[:].rearrange("p (r c) -> p r c", r=16, c=32)
                nc.scalar.activation(half, src, AF.Copy)
        zt = spool.tile([DH, L], BF)
        for h in range(2):
            ps = ppool.tile([DH, 512], F32, tag=f"ping{pp % 2}", name=f"z{h}")
            pp += 1
            nc.tensor.matmul(ps[:], w_z_sb[:],
                             xT_sb[:, h * 512:(h + 1) * 512],
                             start=True, stop=True)
            nc.scalar.activation(zt[:, h * 512:(h + 1) * 512], ps[:], AF.Copy)
            nc.scalar.activation(sg[:, h * 512:(h + 1) * 512], ps[:], AF.Sigmoid)
        nc.vector.tensor_tensor(sg[:], sg[:], zt[:], OP.mult)

        # ---- phase 2: depthwise conv + silu -> xc ----
        xc = [spool.tile([DH, L], BF, name=f"xc{i}") for i in range(2)]
        xcT = [spool.tile([DH, L], BF, name=f"xcT{i}") for i in range(2)]
        for cblk in range(2):
            acc = kpool.tile([DH, L], BF, tag="cacc")
            for tap in range(9):
                dy, dx = tap // 3, tap % 3
                view = xpad[cblk][:, dy * 34 + dx:dy * 34 + dx + 32 * 34]
                view = view.rearrange("p (r c) -> p r c", r=32, c=34)[:, :, 0:32]
                wcol = convw_sb[:, cblk * 9 + tap:cblk * 9 + tap + 1]
                if tap == 0:
                    bcol = convb_sb[:, cblk:cblk + 1]
                    nc.vector.tensor_scalar(nat3(acc[:]), view, wcol, bcol,
                                            OP.mult, OP.add)
                else:
                    acc2 = kpool.tile([DH, L], BF, tag="cacc")
                    nc.vector.scalar_tensor_tensor(
                        nat3(acc2[:]), view, wcol, nat3(acc[:]), OP.mult, OP.add)
                    acc = acc2
            nc.scalar.activation(xc[cblk][:], acc[:], AF.Sigmoid)
            nc.vector.tensor_tensor(xc[cblk][:], xc[cblk][:], acc[:], OP.mult)
            # transposed-sequence copy for the k=1,3 (WH-order) directions:
            # xcT[p, h*32+w] = xc[p, w*32+h]
            nc.vector.tensor_copy(nat3(xcT[cblk][:]), tview(xc[cblk][:]))

        # ---- phase 3: x_dbl, dt, delta, u ----
        # k=0,2 read the row-major xc; k=1,3 read the transposed xcT, so every
        # downstream tensor for those directions is already in WH order and the
        # scan loop never needs strided access.
        du = []     # [DH, 2048] per k: cols 0:1024 delta, 1024:2048 u
        bc_sb = []  # [2*NS, L] per k: B rows then C rows
        for k in range(K):
            xsrc = xc if k in (0, 2) else xcT
            zk = ppool.tile([64, 512], F32, tag="ping0", name="zk")
            zk2 = ppool.tile([64, 512], F32, tag="ping1", name="zk2")
            for h, zz in enumerate((zk, zk2)):
                for cblk in range(2):
                    w0 = (k * 2 + cblk) * 64
                    nc.tensor.matmul(
                        zz[:],
                        xpw_sb[:, w0:w0 + 64],
                        xsrc[cblk][:, h * 512:(h + 1) * 512],
                        start=(cblk == 0), stop=(cblk == 1))
            dts = kpool.tile([RD, L], BF, tag="dts")
            bck = k1pool.tile([2 * NS, L], BF, tag=f"bck{k}")
            for h, zz in enumerate((zk, zk2)):
                nc.scalar.activation(dts[:, h * 512:(h + 1) * 512],
                                     zz[0:RD, :], AF.Copy)
                nc.vector.tensor_copy(bck[:, h * 512:(h + 1) * 512],
                                      zz[32:64, :])
            bc_sb.append(bck)

            dtd = ppool.tile([DH, 512], F32, tag="ping0", name="dtd")
            dtd2 = ppool.tile([DH, 512], F32, tag="ping1", name="dtd2")
            for h, dd in enumerate((dtd, dtd2)):
                nc.tensor.matmul(dd[:], dtw_sb[:, k * DH:(k + 1) * DH],
                                 dts[:, h * 512:(h + 1) * 512],
                                 start=True, stop=True)
            duk = k1pool.tile([DH, 2 * L], BF, tag=f"du{k}")
            esp = kpool.tile([DH, L], F32, tag="esp")
            for h, dd in enumerate((dtd, dtd2)):
                nc.scalar.activation(esp[:, h * 512:(h + 1) * 512], dd[:],
                                     AF.Exp, bias=dtb_sb[:, k:k + 1], scale=1.0)
            # delta = ln(1 + e^(dt+bias)) ; store row-major
            nc.scalar.activation(duk[:, 0:L], esp[:], AF.Ln, bias=1.0, scale=1.0)
            # u = delta * xs_k (xs_k = xc permuted; delta is row-major here,
            # so u is row-major too: u_rm[l] = delta_rm[l] * xc[l])
            du.append(duk)

        # u = delta * xs_k; xs_k for this core's half is xc[0] (row-major
        # dirs) or xcT[0] (WH dirs) — host reorders w_xi so tile 0 is always
        # this core's half.
        for k in range(K):
            xsrc = xc if k in (0, 2) else xcT
            nc.vector.tensor_tensor(du[k][:, L:2 * L], du[k][:, 0:L],
                                    xsrc[0][:], OP.mult)

        # ---- phase 4: per-direction scan (all dense) ----
        # Order [0,2] then [1,3]: the two layout groups share one PSUM
        # accumulator region; the row-major result is drained to SBUF before
        # the WH-order group restarts accumulation.
        y_ps = ypool.tile([DH, L], F32)
        y_rm_sb = spool.tile([DH, L], BF)
        y_wh_sb = spool.tile([DH, L], BF)
        for ki, k in enumerate((0, 2, 1, 3)):
            flip = k >= 2
            bb = kpool.tile([128, L], BF, tag="Bb")
            cb = kpool.tile([128, L], BF, tag="Cb")
            for r in range(8):
                nc.sync.dma_start(bb[16 * r:16 * (r + 1), :],
                                  bc_sb[k][0:NS, :])
                nc.sync.dma_start(cb[16 * r:16 * (r + 1), :],
                                  bc_sb[k][NS:2 * NS, :])
            for t in range(NT):
                bcp = bpool.tile([128, 2 * L], F32, tag="bcp")
                for q in range(4):
                    nc.tensor.matmul(bcp[:, q * 512:(q + 1) * 512],
                                     bcm_sb[:, t * 128:(t + 1) * 128],
                                     du[k][:, q * 512:(q + 1) * 512],
                                     start=True, stop=True)
                a_t = tpool.tile([128, L], BF, tag="a")
                b_t = tpool.tile([128, L], BF, tag="b")
                scl = app_sb[:, k * NT + t:k * NT + t + 1]
                nc.scalar.activation(a_t[:], bcp[:, 0:L],
                                     AF.Exp, bias=0.0, scale=scl)
                nc.vector.scalar_tensor_tensor(
                    b_t[:], bcp[:, L:2 * L], 1.0, bb[:], OP.mult, OP.mult)
                h_t = tpool.tile([128, L], BF, tag="h")
                if flip:
                    nc.vector.tensor_tensor_scan(
                        h_t[:, ::-1], a_t[:, ::-1], b_t[:, ::-1], 0.0,
                        OP.mult, OP.add)
                else:
                    nc.vector.tensor_tensor_scan(
                        h_t[:], a_t[:], b_t[:], 0.0, OP.mult, OP.add)
                hc_t = tpool.tile([128, L], BF, tag="hc")
                nc.vector.tensor_tensor(hc_t[:], h_t[:], cb[:], OP.mult)
                for q in range(2):
                    nc.tensor.matmul(y_ps[:, q * 512:(q + 1) * 512],
                                     red_sb[:, t * DH:(t + 1) * DH],
                                     hc_t[:, q * 512:(q + 1) * 512],
                                     start=(ki % 2 == 0 and t == 0),
                                     stop=(ki % 2 == 1 and t == NT - 1))
            if ki == 1:
                nc.vector.tensor_copy(y_rm_sb[:], y_ps[:])

        # un-transpose the WH-order accumulator back to row-major order
        nc.vector.tensor_copy(nat3(y_wh_sb[:]), tview(y_ps[:]))

        # ---- phase 5: D-term, LN stats, AllReduce ----
        y_full = spool.tile([DH, L], BF)
        nc.vector.scalar_tensor_tensor(y_full[:], xc[0][:], dsum_sb[:],
                                       y_rm_sb[:], OP.mult, OP.add)
        nc.vector.tensor_tensor(y_full[:], y_full[:], y_wh_sb[:], OP.add)
        y2 = spool.tile([DH, L], BF)
        nc.scalar.activation(y2[:], y_full[:], AF.Square)
        st_y = spool.tile([1, L], F32)
        st_y2 = spool.tile([1, L], F32)
        for h in range(2):
            for row, (src_t, dst_t) in enumerate(((y_full, st_y), (y2, st_y2))):
                ssp = ppool.tile([1, 512], F32, tag=f"ping{(2 * h + row) % 2}",
                                 name=f"st{h}{row}")
                nc.tensor.matmul(ssp[:], ones_sb[:, row:row + 1],
                                 src_t[:, h * 512:(h + 1) * 512],
                                 start=True, stop=True)
                nc.scalar.activation(dst_t[:, h * 512:(h + 1) * 512],
                                     ssp[:], AF.Copy)
        nc.sync.dma_start(stats_in[0:1, :], st_y[:])
        nc.sync.dma_start(stats_in[1:2, :], st_y2[:])
        nc.gpsimd.collective_compute(
            "AllReduce", OP.add, replica_groups=groups,
            ins=[stats_in[:]], outs=[stats_out[:]])
        # reshape [2, 1024] -> [128, 16] straight from DRAM
        st128 = spool.tile([128, 16], F32)
        nc.sync.dma_start(
            st128[:].rearrange("p (s f) -> p s f", s=2, f=8),
            stats_out[:].rearrange("s (p f) -> p s f", p=128, f=8))
        mu8 = spool.tile([128, 8], F32)
        nc.scalar.activation(mu8[:], st128[:, 0:8], AF.Copy, scale=1.0 / DI)
        msq = spool.tile([128, 8], F32)
        nc.scalar.activation(msq[:], st128[:, 0:8], AF.Square, scale=1.0 / DI)
        var8 = spool.tile([128, 8], F32)
        nc.vector.scalar_tensor_tensor(var8[:], st128[:, 8:16], 1.0 / DI,
                                       msq[:], OP.mult, OP.subtract)
        eps_sb = spool.tile([128, 1], F32)
        nc.vector.memset(eps_sb[:], EPS)
        sd8 = spool.tile([128, 8], F32)
        nc.scalar.activation(sd8[:], var8[:], AF.Sqrt, bias=eps_sb[:], scale=1.0)
        inv8 = spool.tile([128, 8], F32)
        nc.vector.reciprocal(inv8[:], sd8[:])
        nc.sync.dma_start(
            minv_dram[0:1, :].rearrange("s (p f) -> p s f", p=128, f=8),
            mu8[:].unsqueeze(1))
        nc.sync.dma_start(
            minv_dram[1:2, :].rearrange("s (p f) -> p s f", p=128, f=8),
            inv8[:].unsqueeze(1))
        minv = spool.tile([2, L], F32)
        nc.sync.dma_start(minv[:], minv_dram[:])

        # broadcast mu/inv across partitions via PE (contraction dim 2)
        t1 = spool.tile([DH, L], BF)
        for h in range(2):
            mm = ppool.tile([DH, 512], F32, tag=f"ping{h % 2}", name=f"mu{h}")
            nc.tensor.matmul(mm[:], sel2_sb[:, 0:DH],
                             minv[:, h * 512:(h + 1) * 512],
                             start=True, stop=True)
            nc.vector.tensor_tensor(t1[:, h * 512:(h + 1) * 512],
                                    y_full[:, h * 512:(h + 1) * 512],
                                    mm[:], OP.subtract)
        t2 = spool.tile([DH, L], BF)
        for h in range(2):
            ii = ppool.tile([DH, 512], F32, tag=f"ping{h % 2}", name=f"iv{h}")
            nc.tensor.matmul(ii[:], sel2_sb[:, DH:2 * DH],
                             minv[:, h * 512:(h + 1) * 512],
                             start=True, stop=True)
            nc.vector.scalar_tensor_tensor(t2[:, h * 512:(h + 1) * 512],
                                           t1[:, h * 512:(h + 1) * 512],
                                           gam_sb[:], ii[:], OP.mult, OP.mult)
        t3 = spool.tile([DH, L], BF)
        nc.vector.scalar_tensor_tensor(t3[:], t2[:], bet_sb[:], sg[:],
                                       OP.add, OP.mult)

        o_sb = spool.tile([DM, L], F32)
        for h in range(2):
            oo = ppool.tile([DM, 512], F32, tag=f"ping{h % 2}", name=f"o{h}")
            nc.tensor.matmul(oo[:], wout_sb[:],
                             t3[:, h * 512:(h + 1) * 512],
                             start=True, stop=True)
            nc.scalar.activation(o_sb[:, h * 512:(h + 1) * 512], oo[:], AF.Copy)
        nc.sync.dma_start(out_part[:], o_sb[:])

    nc.finalize()
    return nc


def _prep_inputs(inputs):
    """Build the 8 per-core input maps. Core c: b = c//2, dh = c%2."""
    x = np.asarray(inputs["x"], np.float32)
    in_proj_w = np.asarray(inputs["in_proj_w"], np.float32)
    conv_w = np.asarray(inputs["conv_w"], np.float32)
    conv_b = np.asarray(inputs["conv_b"], np.float32)
    xpw = np.asarray(inputs["x_proj_weight"], np.float32)
    dtw = np.asarray(inputs["dt_projs_weight"], np.float32)
    dtb = np.asarray(inputs["dt_projs_bias"], np.float32)
    A_logs = np.asarray(inputs["A_logs"], np.float32)
    Ds = np.asarray(inputs["Ds"], np.float32)
    gam = np.asarray(inputs["ln_gamma"], np.float32)
    bet = np.asarray(inputs["ln_beta"], np.float32)
    wout = np.asarray(inputs["out_proj_w"], np.float32)

    xTf = x.reshape(B, L, DM).transpose(0, 2, 1).copy()      # (B, 96, 1024)
    w_in_T = in_proj_w.T.copy()                               # (96, 384)
    convw9 = conv_w.reshape(DI, 9)                            # (192, 9)
    A = -np.exp(A_logs).reshape(K, DI, NS)                    # (K, 192, 16)
    Dsum_full = Ds.reshape(K, DI).sum(0)                      # (192,)

    bcm = np.zeros((DH, NT * 128), np.float32)
    for t in range(NT):
        for j in range(128):
            bcm[8 * t + j // 16, t * 128 + j] = 1.0
    red = np.zeros((128, NT * DH), np.float32)
    for t in range(NT):
        for j in range(128):
            red[j, t * DH + 8 * t + j // 16] = 1.0
    ones96 = np.ones((DH, 2), np.float32)
    sel2 = np.zeros((2, 2 * DH), np.float32)
    sel2[0, 0:DH] = 1.0
    sel2[1, DH:2 * DH] = 1.0

    in_maps = []
    for c in range(8):
        b, dh = c // 2, c % 2
        ds = slice(dh * DH, (dh + 1) * DH)
        other = slice((1 - dh) * DH, (2 - dh) * DH)
        # xc tile 0 must hold THIS core's half: reorder in_proj rows and
        # x_dbl contraction rows to match (half-first ordering).
        w_xi = np.concatenate([w_in_T[:, ds], w_in_T[:, other]], axis=1)
        convw_r = np.concatenate([convw9[ds], convw9[other]], axis=1)
        convb_r = np.stack([conv_b[ds], conv_b[other]], axis=1)
        xpw_r = np.zeros((DH, K * 2 * 64), np.float32)
        for k in range(K):
            wk = xpw[k].T  # (192, 38)
            for cblk, sl in enumerate((ds, other)):
                w0 = (k * 2 + cblk) * 64
                xpw_r[:, w0:w0 + RD] = wk[sl][:, 0:RD]
                xpw_r[:, w0 + 32:w0 + 64] = wk[sl][:, RD:RD + 2 * NS]
        dtw_r = np.zeros((RD, K * DH), np.float32)
        for k in range(K):
            dtw_r[:, k * DH:(k + 1) * DH] = dtw[k, ds, :].T
        dtb_r = dtb.reshape(K, DI)[:, ds].T.copy()            # (96, K)
        app = np.zeros((128, K * NT), np.float32)
        for k in range(K):
            for t in range(NT):
                for j in range(128):
                    app[j, k * NT + t] = A[k, dh * DH + 8 * t + j // 16, j % 16]
        in_maps.append({
            "xT": xTf[b].astype(BF_NP),
            "w_xi": w_xi.astype(BF_NP),
            "w_z": w_in_T[:, DI + dh * DH: DI + (dh + 1) * DH].astype(BF_NP),
            "convw": convw_r,
            "convb": convb_r,
            "xpw": xpw_r.astype(BF_NP),
            "dtw": dtw_r.astype(BF_NP),
            "dtb": dtb_r,
            "app": app,
            "bcm": bcm.astype(BF_NP),
            "red": red.astype(BF_NP),
            "dsum": Dsum_full[ds][:, None],
            "gam": gam[ds][:, None],
            "bet": bet[ds][:, None],
            "wout": wout[:, ds].T.astype(BF_NP),
            "ones96": ones96.astype(BF_NP),
            "sel2": sel2,
        })
    return in_maps


def kernel(**inputs):
    global _NC
    if _NC is None:
        _NC = build()
    in_maps = _prep_inputs(inputs)
    res = run_bass_kernel_spmd(_NC, in_maps, list(range(8)))
    out = np.zeros((B, L, DM), np.float32)
    for b in range(B):
        part = res.results[2 * b]["out_part"] + res.results[2 * b + 1]["out_part"]
        out[b] = part.T
    return out.reshape(B, HH, WW, DM)

